# revision 1
# baseline (speedup 1.0000x reference)
"""Multi-head causal attention (B=4, S=2048, D=1024, H=16) on 8 NeuronCores.

Sharding: core c -> batch b = c//2, head-group g = c%2 (8 heads each).
Each core computes, for its batch and heads:
    V     = x @ Wv             (natural layout, plus ones column for denom)
    QT/KT = W.T @ x.T          (transposed projections, [64, S] per head)
    ST    = K_chunk @ Q_blk.T  ([k=128, q=512] score chunks, causal-skipped)
    E     = exp(ST/8) (* triangle mask on the partial diagonal block)
    accT  = V_aug.T @ E        ([65, q]: rows 0-63 unnormalized out.T, row 64 denom)
    out.T = accT[:64] / accT[64]  (stacked over heads -> concatT [512, S])
    y_part = concatT.T @ W_O_part.T
Host sums the two partial y's per batch (the "all-reduce after W_O").

All matmuls run as float32r (full fp32 data, 1 cycle/row PE mode).
"""

import numpy as np

import concourse.bass as bass
import concourse.tile as tile
import concourse.mybir as mybir
from concourse import bacc
from concourse.bass_utils import run_bass_kernel_spmd

B, S, D, H, HD = 4, 2048, 1024, 16, 64
NH = 8            # heads per core
NP = NH // 2      # head pairs per core
QB = 512          # q block size
NQB = S // QB     # 4
KC = 128          # k chunk size
ND = QB // KC     # diagonal chunks per q block
NKT = D // 128    # 8 contraction tiles over D
NST = S // 128    # 16 s tiles
CW = NH * HD      # 512 concat width per core
DW = QB + (QB - KC) + (QB - 2 * KC) + (QB - 3 * KC)  # 1280 packed diag width

F32 = mybir.dt.float32
F32R = mybir.dt.float32r
AF = mybir.ActivationFunctionType

N_CORES = 8

_cache = {}


def _r(ap):
    return ap.bitcast(F32R)


def build_nc(repeats=1, phases="full", hw_loop=False):
    nc = bacc.Bacc("TRN2", target_bir_lowering=False, debug=False,
                   num_devices=N_CORES)
    x_ck = nc.dram_tensor("x_ck", [NQB, 128, NKT, QB], F32R,
                          kind="ExternalInput").ap()
    wq = nc.dram_tensor("wq", [128, NKT, CW], F32R, kind="ExternalInput").ap()
    wk = nc.dram_tensor("wk", [128, NKT, CW], F32R, kind="ExternalInput").ap()
    wv = nc.dram_tensor("wv", [128, NKT, CW], F32R, kind="ExternalInput").ap()
    wot = nc.dram_tensor("wot", [128, CW // 128, D], F32R,
                         kind="ExternalInput").ap()
    masks = nc.dram_tensor("masks", [KC, KC], F32R, kind="ExternalInput").ap()
    ones = nc.dram_tensor("ones", [128, NST * NH], F32R,
                          kind="ExternalInput").ap()
    y = nc.dram_tensor("y", [S, D], F32, kind="ExternalOutput").ap()

    with tile.TileContext(nc) as tc:
        if hw_loop:
            with tc.For_i(0, repeats, 1):
                _build(tc, x_ck, wq, wk, wv, wot, masks, ones, y, phases)
        else:
            for _ in range(repeats):
                _build(tc, x_ck, wq, wk, wv, wot, masks, ones, y, phases)
    nc.compile()
    return nc


def _build(tc, x_ck, wq, wk, wv, wot, masks, ones, y, phases="full"):
    nc = tc.nc
    with tc.tile_pool(name="persist", bufs=1) as persist:
        qt_sb = persist.tile([128, NP, S], F32R)      # [2 heads, pair, s]
        kt_sb = persist.tile([128, NP, S], F32R)
        v_sb = persist.tile([128, NST, NH, HD + 1], F32R)
        tri_sb = persist.tile([128, KC], F32R)
        wot_sb = persist.tile([128, CW // 128, D], F32R)
        nc.sync.dma_start(tri_sb, masks)
        nc.sync.dma_start(wot_sb, wot)
        v_ones = bass.AP(tensor=v_sb.tensor, offset=v_sb.offset + HD,
                         ap=[list(v_sb.ap[0]), [HD + 1, NST * NH], [1, 1]])
        nc.sync.dma_start(v_ones, ones.rearrange("p (n o) -> p n o", o=1))

        # ---- projections: pass A computes Q and V from streamed x chunks,
        # ---- pass B computes K from a second stream of the same chunks
        with (
            tc.tile_pool(name="qk_w", bufs=1) as qkw,
            tc.tile_pool(name="qk_s", bufs=2) as qks,
            tc.tile_pool(name="qk_p", bufs=1, space="PSUM") as qkp,
        ):
            wq_sb = qkw.tile([128, NKT, CW], F32R)
            wk_sb = qkw.tile([128, NKT, CW], F32R)
            wv_sb = qkw.tile([128, NKT, CW], F32R)
            nc.sync.dma_start(wq_sb, wq)
            nc.sync.dma_start(wk_sb, wk)
            nc.sync.dma_start(wv_sb, wv)
            run_a = phases not in ("qk", "dma")
            run_qk = phases not in ("v", "dma")
            for c in range(NQB):
                psq = [qkp.tile([128, QB], F32, tag=f"psq{p}", name=f"psq{p}")
                       for p in range(NP)]
                psv = [qkp.tile([128, CW], F32, tag=f"psv{i}", name=f"psv{i}")
                       for i in range(4)]
                xs = qks.tile([128, NKT, QB], F32R, tag="xs")
                nc.sync.dma_start(xs, x_ck[c])
                for k in range(NKT):
                    if run_qk:
                        for p in range(NP):
                            nc.tensor.matmul(
                                psq[p], _r(wq_sb[:, k, p * 128:(p + 1) * 128]),
                                _r(xs[:, k, :]),
                                start=(k == 0), stop=(k == NKT - 1))
                    if run_a:
                        for i in range(4):
                            nc.tensor.matmul(
                                psv[i], _r(xs[:, k, i * 128:(i + 1) * 128]),
                                _r(wv_sb[:, k, :]),
                                start=(k == 0), stop=(k == NKT - 1))
                if run_qk:
                    for p in range(NP):
                        nc.vector.tensor_copy(
                            qt_sb[:, p, c * QB:(c + 1) * QB], psq[p])
                if run_a:
                    for i in range(4):
                        nc.vector.tensor_copy(
                            v_sb[:, c * 4 + i, :, 0:HD],
                            psv[i].rearrange("p (h e) -> p h e", h=NH))
            for c in range(NQB if run_qk else 0):
                psk = [qkp.tile([128, QB], F32, tag=f"psq{p}", name=f"psk{p}")
                       for p in range(NP)]
                xs = qks.tile([128, NKT, QB], F32R, tag="xs")
                nc.sync.dma_start(xs, x_ck[c])
                for k in range(NKT):
                    for p in range(NP):
                        nc.tensor.matmul(
                            psk[p], _r(wk_sb[:, k, p * 128:(p + 1) * 128]),
                            _r(xs[:, k, :]),
                            start=(k == 0), stop=(k == NKT - 1))
                for p in range(NP):
                    nc.vector.tensor_copy(
                        kt_sb[:, p, c * QB:(c + 1) * QB], psk[p])

        if phases == "dma":
            with tc.tile_pool(name="dma_s", bufs=2) as dms:
                for t in range(NST):
                    for nh_ in range(2):
                        ysb = dms.tile([128, 512], F32R, tag="ysb")
                        nc.vector.tensor_copy(
                            ysb, wot_sb[:, nh_, (t % 2) * 512:(t % 2 + 1) * 512])
                        nc.sync.dma_start(
                            y[t * 128:(t + 1) * 128,
                              nh_ * 512:(nh_ + 1) * 512].bitcast(F32R), ysb)
            return

        if phases in ("vqk", "qk", "v"):
            # truncated build for HW bisection: write qt/kt straight out
            with tc.tile_pool(name="tr_s", bufs=2) as trs:
                for c in range(NQB):
                    tr = trs.tile([128, QB], F32R, tag="tr")
                    if phases == "v":
                        nc.vector.tensor_copy(
                            tr, v_sb[:, c, :, :].rearrange(
                                "p h e -> p (h e)")[:, 0:QB])
                    else:
                        nc.vector.tensor_mul(
                            tr, qt_sb[:, 0, c * QB:(c + 1) * QB],
                            kt_sb[:, 0, c * QB:(c + 1) * QB])
                    nc.sync.dma_start(
                        y[c * 128:(c + 1) * 128, 0:QB].bitcast(F32R), tr)
            return

        # ---- attention, with W_O interleaved per q block ------------------
        with (
            tc.tile_pool(name="at_c", bufs=1) as atc,
            tc.tile_pool(name="at_e", bufs=3) as ate,
            tc.tile_pool(name="at_d", bufs=2) as atd,
            tc.tile_pool(name="at_r", bufs=2) as atr,
            tc.tile_pool(name="wo_s", bufs=2) as wos,
            tc.tile_pool(name="at_pp", bufs=2, space="PSUM") as atpp,
            tc.tile_pool(name="at_po", bufs=2, space="PSUM") as atpo,
            tc.tile_pool(name="wo_p", bufs=2, space="PSUM") as wop,
        ):
            concat_sb = atc.tile([128, NP, S], F32R)
            for qb in range(NQB):
                nkc = (qb + 1) * ND
                qsl = slice(qb * QB, (qb + 1) * QB)
                for h in range(NH):
                    p, r0 = h // 2, 64 * (h % 2)
                    pso = atpo.tile([HD + 1, QB], F32, tag="pso")
                    # full (below-diagonal) chunks, in pairs: one Exp spans
                    # 1024 columns to amortize the ACT per-op overhead
                    for kc0 in range(0, nkc - ND, 2):
                        pp = atpp.tile([128, 2, QB], F32, tag="pp")
                        for i in range(2):
                            kc = kc0 + i
                            nc.tensor.matmul(
                                pp[:, i, :],
                                _r(kt_sb[r0:r0 + 64, p,
                                         kc * KC:(kc + 1) * KC]),
                                _r(qt_sb[r0:r0 + 64, p, qsl]),
                                start=True, stop=True)
                        ex2 = ate.tile([128, 2, QB], F32R, tag="ex2")
                        nc.scalar.activation(ex2, pp, AF.Exp, scale=0.125)
                        for i in range(2):
                            kc = kc0 + i
                            nc.tensor.matmul(
                                pso, _r(v_sb[:, kc, h, :]), _r(ex2[:, i, :]),
                                start=(kc == 0), stop=False)
                    # diagonal chunks: exp on the live column subrange from
                    # PSUM, triangle mask on the partial 128x128 block
                    for jp in range(ND // 2):
                        pp = atpp.tile([128, 2, QB], F32, tag="pp",
                                       name="ppd")
                        for i in range(2):
                            j = jp * 2 + i
                            kc = nkc - ND + j
                            q0 = j * KC
                            nc.tensor.matmul(
                                pp[:, i, q0:],
                                _r(kt_sb[r0:r0 + 64, p,
                                         kc * KC:(kc + 1) * KC]),
                                _r(qt_sb[r0:r0 + 64, p,
                                         qb * QB + q0:(qb + 1) * QB]),
                                start=True, stop=True)
                        for i in range(2):
                            j = jp * 2 + i
                            kc = nkc - ND + j
                            q0 = j * KC
                            ex = ate.tile([128, QB], F32R, tag="ex")
                            nc.scalar.activation(ex[:, q0:], pp[:, i, q0:],
                                                 AF.Exp, scale=0.125)
                            nc.vector.tensor_mul(ex[:, q0:q0 + KC],
                                                 ex[:, q0:q0 + KC], tri_sb)
                            nc.tensor.matmul(
                                pso[:, q0:], _r(v_sb[:, kc, h, :]),
                                _r(ex[:, q0:]),
                                start=(kc == 0), stop=(kc == nkc - 1))
                    # normalize: divide rows 0-63 by the denominator row
                    recip = atr.tile([1, QB], F32, tag="recip")
                    nc.vector.reciprocal(recip, pso[HD:HD + 1, :])
                    recip_b = atr.tile([64, QB], F32, tag="recip_b")
                    nc.gpsimd.partition_broadcast(recip_b, recip)
                    nc.vector.tensor_mul(
                        concat_sb[r0:r0 + 64, p, qsl], pso[0:HD, :], recip_b)

                # W_O for the q rows finished by this q block
                for t in range(qb * QB // 128, (qb + 1) * QB // 128):
                    ysb = wos.tile([128, D], F32, tag="ysb")
                    for nh_ in range(2):
                        psy = wop.tile([128, 512], F32, tag="psy")
                        for cc in range(CW // 128):
                            nc.tensor.matmul(
                                psy,
                                _r(concat_sb[:, cc, t * 128:(t + 1) * 128]),
                                _r(wot_sb[:, cc, nh_ * 512:(nh_ + 1) * 512]),
                                start=(cc == 0), stop=(cc == CW // 128 - 1))
                        nc.vector.tensor_copy(
                            ysb[:, nh_ * 512:(nh_ + 1) * 512], psy)
                    nc.sync.dma_start(y[t * 128:(t + 1) * 128, :], ysb)


def shard_inputs(x, Wq, Wk, Wv, W_O):
    """Build the 8 per-core input maps from full inputs."""
    masks = (np.arange(KC)[:, None] <= np.arange(KC)[None, :]).astype(
        np.float32)

    def wtile(w):
        # [D, CW] -> [128, NKT, CW] with row d = k*128 + p
        return np.ascontiguousarray(w.reshape(NKT, 128, CW).transpose(1, 0, 2))

    in_maps = []
    for c in range(N_CORES):
        b, g = c // 2, c % 2
        hs = slice(g * NH, (g + 1) * NH)
        xT = np.ascontiguousarray(x[b].T)
        x_ck = np.ascontiguousarray(
            xT.reshape(NKT, 128, NQB, QB).transpose(2, 1, 0, 3))
        wot = np.ascontiguousarray(W_O[:, g * CW:(g + 1) * CW].T)
        in_maps.append({
            "x_ck": x_ck,
            "wq": wtile(Wq[hs].transpose(1, 0, 2).reshape(D, CW)),
            "wk": wtile(Wk[hs].transpose(1, 0, 2).reshape(D, CW)),
            "wv": wtile(Wv[hs].transpose(1, 0, 2).reshape(D, CW)),
            "wot": np.ascontiguousarray(
                wot.reshape(CW // 128, 128, D).transpose(1, 0, 2)),
            "masks": masks,
            "ones": np.ones((128, NST * NH), np.float32),
        })
    return in_maps


def kernel(x, Wq, Wk, Wv, W_O):
    x = np.asarray(x, np.float32)
    Wq = np.asarray(Wq, np.float32)
    Wk = np.asarray(Wk, np.float32)
    Wv = np.asarray(Wv, np.float32)
    W_O = np.asarray(W_O, np.float32)

    if "nc" not in _cache:
        _cache["nc"] = build_nc()
    nc = _cache["nc"]

    in_maps = shard_inputs(x, Wq, Wk, Wv, W_O)
    res = run_bass_kernel_spmd(nc, in_maps, core_ids=list(range(N_CORES)))
    _cache["last_results"] = res

    y = np.zeros((B, S, D), np.float32)
    for c in range(N_CORES):
        y[c // 2] += res.results[c]["y"]
    return y

